# revision 1
# baseline (speedup 1.0000x reference)
"""Trainium2 Bass kernel for nn_MultiHeadRelationalModule.

Data-parallel over batch across 8 NeuronCores. The device kernel computes
the dense per-token pipeline (1x1 conv1 -> relu -> 1x1 conv2 -> relu ->
K/Q/V projections with coordinate-channel + bias folded in) in
feature-major layout with PE matmuls. The remaining small attention tail
is computed on host in fp32 numpy.
"""
import numpy as np
from contextlib import ExitStack

import concourse.bacc as bacc
import concourse.bass as bass
import concourse.tile as tile
from concourse import mybir
from concourse.bass_utils import run_bass_kernel_spmd

N_CORES = 8
B = 8192
B_LOC = B // N_CORES          # 1024
NODES = 49
ROWS = B_LOC * NODES          # 50176
NHEADS, D = 3, 64
EPS = 1e-5

CHUNK_B = 10                  # batch elems per matmul stream chunk
CHUNK = CHUNK_B * NODES       # 490 cols, fits one PSUM bank (<=512 f32)
# feature blocks of the 576-wide kqv projection
FEAT_BLOCKS = [(0, 128), (128, 128), (256, 128), (384, 128), (512, 64)]

_CACHE = {}


def _build_nc():
    nc = bacc.Bacc(None, target_bir_lowering=False)
    dt = mybir.dt.float32
    xt_d = nc.dram_tensor("xt", [3, ROWS], dt, kind="ExternalInput")
    w1_d = nc.dram_tensor("w1t", [3, 16], dt, kind="ExternalInput")
    b1_d = nc.dram_tensor("b1", [16, 1], dt, kind="ExternalInput")
    w2_d = nc.dram_tensor("w2t", [16, 20], dt, kind="ExternalInput")
    b2_d = nc.dram_tensor("b2", [20, 1], dt, kind="ExternalInput")
    wp_d = nc.dram_tensor("wp", [20, 576], dt, kind="ExternalInput")
    cc_d = nc.dram_tensor("cc", [576, NODES], dt, kind="ExternalInput")
    out_d = nc.dram_tensor("kqvt", [576, ROWS], dt, kind="ExternalOutput")

    with tile.TileContext(nc) as tc, ExitStack() as ctx:
        singles = ctx.enter_context(tc.tile_pool(name="singles", bufs=1))
        xpool = ctx.enter_context(tc.tile_pool(name="xin", bufs=3))
        hpool = ctx.enter_context(tc.tile_pool(name="hbuf", bufs=3))
        opool = ctx.enter_context(tc.tile_pool(name="obuf", bufs=3))
        psum = ctx.enter_context(tc.tile_pool(name="ps", bufs=2, space="PSUM"))
        psum2 = ctx.enter_context(tc.tile_pool(name="ps2", bufs=2, space="PSUM"))

        w1_s = singles.tile([3, 16], dt)
        nc.sync.dma_start(w1_s[:], w1_d[:])
        b1_s = singles.tile([16, 1], dt)
        nc.sync.dma_start(b1_s[:], b1_d[:])
        w2_s = singles.tile([16, 20], dt)
        nc.sync.dma_start(w2_s[:], w2_d[:])
        b2_s = singles.tile([20, 1], dt)
        nc.sync.dma_start(b2_s[:], b2_d[:])
        wp_s = singles.tile([20, 576], dt)
        nc.sync.dma_start(wp_s[:], wp_d[:])
        # coordinate+bias contribution, replicated along the chunk's batch dim
        cc_rep = []
        for bi, (f0, fn) in enumerate(FEAT_BLOCKS):
            t = singles.tile([fn, CHUNK], dt, tag=f"ccrep{bi}")
            src = bass.AP(
                tensor=cc_d.tensor if hasattr(cc_d, "tensor") else cc_d,
                offset=f0 * NODES,
                ap=[[NODES, fn], [0, CHUNK_B], [1, NODES]],
            )
            nc.sync.dma_start(t[:], src)
            cc_rep.append(t)

        n_full = B_LOC // CHUNK_B            # 102 full chunks
        rem_b = B_LOC - n_full * CHUNK_B     # 4
        spans = [(i * CHUNK, CHUNK) for i in range(n_full)]
        if rem_b:
            spans.append((n_full * CHUNK, rem_b * NODES))

        for c0, w in spans:
            xt_t = xpool.tile([3, CHUNK], dt, tag="xt")
            nc.sync.dma_start(xt_t[:, :w], xt_d[:, c0:c0 + w])

            h1_ps = psum.tile([16, CHUNK], dt, tag="h1ps")
            nc.tensor.matmul(h1_ps[:, :w], w1_s[:], xt_t[:, :w],
                             start=True, stop=True)
            h1_s = hpool.tile([16, CHUNK], dt, tag="h1")
            nc.scalar.activation(h1_s[:, :w], h1_ps[:, :w],
                                 mybir.ActivationFunctionType.Relu,
                                 bias=b1_s[:], scale=1.0)

            h2_ps = psum.tile([20, CHUNK], dt, tag="h2ps")
            nc.tensor.matmul(h2_ps[:, :w], w2_s[:], h1_s[:, :w],
                             start=True, stop=True)
            h2_s = hpool.tile([20, CHUNK], dt, tag="h2")
            nc.scalar.activation(h2_s[:, :w], h2_ps[:, :w],
                                 mybir.ActivationFunctionType.Relu,
                                 bias=b2_s[:], scale=1.0)

            for bi, (f0, fn) in enumerate(FEAT_BLOCKS):
                p_ps = psum2.tile([fn, CHUNK], dt, tag=f"pps{bi % 2}")
                nc.tensor.matmul(p_ps[:, :w], wp_s[:, f0:f0 + fn],
                                 h2_s[:, :w], start=True, stop=True)
                o_s = opool.tile([fn, CHUNK], dt, tag=f"ob{bi % 2}")
                nc.vector.tensor_add(o_s[:, :w], p_ps[:, :w],
                                     cc_rep[bi][:, :w])
                nc.sync.dma_start(out_d[f0:f0 + fn, c0:c0 + w], o_s[:, :w])
    nc.finalize()
    return nc


def kernel(x, conv1_w, conv1_b, conv2_w, conv2_b,
           k_proj_w, k_proj_b, q_proj_w, q_proj_b, v_proj_w, v_proj_b,
           k_norm_g, k_norm_b, q_norm_g, q_norm_b, v_norm_g, v_norm_b,
           k_lin_w, k_lin_b, q_lin_w, q_lin_b, a_lin_w, a_lin_b,
           lin1_w, lin1_b, lin2_w, lin2_b):
    f32 = np.float32
    x = np.asarray(x, f32)
    b = x.shape[0]

    if "nc" not in _CACHE:
        _CACHE["nc"] = _build_nc()
    nc = _CACHE["nc"]

    # host-side prep of tiny weight tensors
    w1t = np.ascontiguousarray(np.asarray(conv1_w, f32).T)        # [3,16]
    w2t = np.ascontiguousarray(np.asarray(conv2_w, f32).T)        # [16,20]
    wp_full = np.concatenate([np.asarray(k_proj_w, f32),
                              np.asarray(q_proj_w, f32),
                              np.asarray(v_proj_w, f32)], axis=1)  # [22,576]
    wp = np.ascontiguousarray(wp_full[:20])                        # [20,576]
    # coordinate channels (match reference)
    xc = np.tile((np.arange(7, dtype=f32) / 7)[None, :], (7, 1))
    yc = np.tile((np.arange(7, dtype=f32) / 7)[:, None], (1, 7))
    coords = np.stack([xc.reshape(-1), yc.reshape(-1)], axis=1)    # [49,2]
    bias_full = np.concatenate([np.asarray(k_proj_b, f32),
                                np.asarray(q_proj_b, f32),
                                np.asarray(v_proj_b, f32)])        # [576]
    cc = (coords @ wp_full[20:22] + bias_full[None, :]).T          # [576,49]
    cc = np.ascontiguousarray(cc, f32)

    xr = x.reshape(b, 3, NODES)
    in_maps = []
    for c in range(N_CORES):
        xs = xr[c * B_LOC:(c + 1) * B_LOC]                 # [1024,3,49]
        xt = np.ascontiguousarray(
            xs.transpose(1, 0, 2).reshape(3, ROWS), f32)
        in_maps.append({
            "xt": xt, "w1t": w1t, "b1": np.asarray(conv1_b, f32)[:, None],
            "w2t": w2t, "b2": np.asarray(conv2_b, f32)[:, None],
            "wp": wp, "cc": cc,
        })

    res = run_bass_kernel_spmd(nc, in_maps, list(range(N_CORES)))
    kqv = np.concatenate(
        [res.results[c]["kqvt"].T.reshape(B_LOC, NODES, 576)
         for c in range(N_CORES)], axis=0)                 # [B,49,576]

    # ---- host tail (small ops) ----
    def ln(t, axes, g, beta):
        m = t.mean(axis=axes, keepdims=True)
        v = t.var(axis=axes, keepdims=True)
        y = (t - m) / np.sqrt(v + EPS)
        return y * g + beta

    def heads(p):
        return p.reshape(b, NODES, NHEADS, D).transpose(0, 2, 1, 3)

    K = ln(heads(kqv[..., 0:192]), (1, 2, 3), np.asarray(k_norm_g, f32),
           np.asarray(k_norm_b, f32))
    Q = ln(heads(kqv[..., 192:384]), (1, 2, 3), np.asarray(q_norm_g, f32),
           np.asarray(q_norm_b, f32))
    V = ln(heads(kqv[..., 384:576]), (1, 2, 3), np.asarray(v_norm_g, f32),
           np.asarray(v_norm_b, f32))

    def elu(t):
        return np.where(t > 0, t, np.expm1(np.minimum(t, 0.0)))

    A = elu((Q @ np.asarray(q_lin_w, f32) + np.asarray(q_lin_b, f32))
            + (K @ np.asarray(k_lin_w, f32) + np.asarray(k_lin_b, f32)))
    A = A @ np.asarray(a_lin_w, f32) + np.asarray(a_lin_b, f32)
    A = A - A.max(axis=-1, keepdims=True)
    np.exp(A, out=A)
    A /= A.sum(axis=-1, keepdims=True)

    E = A @ V                                              # [B,H,N,D]
    E = E.transpose(0, 2, 1, 3).reshape(b, NODES, NHEADS * D)
    E = np.maximum(E @ np.asarray(lin1_w, f32) + np.asarray(lin1_b, f32), 0.0)
    m = E.mean(axis=(1, 2), keepdims=True)
    v = E.var(axis=(1, 2), keepdims=True)
    E = (E - m) / np.sqrt(v + EPS)
    E = E.max(axis=1)                                      # [B,D]
    out = E @ np.asarray(lin2_w, f32) + np.asarray(lin2_b, f32)
    return elu(out).astype(np.float32)



# revision 2
# speedup vs baseline: 1.0583x; 1.0583x over previous
"""Trainium2 Bass kernel for nn_MultiHeadRelationalModule — full on-device
pipeline, data-parallel over batch across 8 NeuronCores.

Per core: 1024 batch elems, processed in 128 chunks of 8 via a hardware
For_i loop. Feature-major matmuls on PE; layernorm statistics via
PE ones-matmul column sums + DVE grouped reduces; per-batch scalars
broadcast across partitions with k=1 PE matmuls; the V layernorm is
folded algebraically through the attention+lin1 matmuls (valid because
v_norm_g==1, v_norm_b==0, and softmax rows sum to 1).

Only x streams in (602KB/core) and the [5,1024] result streams out.
"""
import numpy as np
from contextlib import ExitStack

import concourse.bacc as bacc
import concourse.bass as bass
import concourse.tile as tile
from concourse import mybir
from concourse.bass import ts
from concourse.bass_utils import run_bass_kernel_spmd

N_CORES = 8
B = 8192
B_LOC = B // N_CORES          # 1024
NODES = 49
NHEADS, D = 3, 64
EPS = 1e-5
CB = 8                        # batch elems per chunk
COLS = CB * NODES             # 392
NCHUNK = B_LOC // CB          # 128
KQV_N = 192                   # NHEADS*D per tensor

_CACHE = {}

AF = mybir.ActivationFunctionType
ALU = mybir.AluOpType


def _build_nc():
    nc = bacc.Bacc(None, target_bir_lowering=False)
    dt = mybir.dt.float32

    xt_d = nc.dram_tensor("xt", [3, B_LOC * NODES], dt, kind="ExternalInput")
    w1_d = nc.dram_tensor("w1t", [3, 16], dt, kind="ExternalInput")
    b1_d = nc.dram_tensor("b1", [16, 1], dt, kind="ExternalInput")
    w2_d = nc.dram_tensor("w2t", [16, 20], dt, kind="ExternalInput")
    b2_d = nc.dram_tensor("b2", [20, 1], dt, kind="ExternalInput")
    wk_d = nc.dram_tensor("wk", [20, KQV_N], dt, kind="ExternalInput")
    wq_d = nc.dram_tensor("wq", [20, KQV_N], dt, kind="ExternalInput")
    wv_d = nc.dram_tensor("wv", [20, KQV_N], dt, kind="ExternalInput")
    ck_d = nc.dram_tensor("ck", [KQV_N, NODES], dt, kind="ExternalInput")
    cq_d = nc.dram_tensor("cq", [KQV_N, NODES], dt, kind="ExternalInput")
    cv_d = nc.dram_tensor("cvtm", [NODES, KQV_N], dt, kind="ExternalInput")
    qlw_d = nc.dram_tensor("qlw", [D, NODES], dt, kind="ExternalInput")
    klw_d = nc.dram_tensor("klw", [D, NODES], dt, kind="ExternalInput")
    qklb_d = nc.dram_tensor("qklb", [NODES, 1], dt, kind="ExternalInput")
    alw_d = nc.dram_tensor("alw", [NODES, NODES], dt, kind="ExternalInput")
    alb_d = nc.dram_tensor("alb", [NODES, 1], dt, kind="ExternalInput")
    l1t_d = nc.dram_tensor("l1t", [D, KQV_N], dt, kind="ExternalInput")
    nls_d = nc.dram_tensor("negl1sum", [1, D], dt, kind="ExternalInput")
    l1b_d = nc.dram_tensor("l1b", [D, 1], dt, kind="ExternalInput")
    l2w_d = nc.dram_tensor("l2w", [D, 5], dt, kind="ExternalInput")
    l2b_d = nc.dram_tensor("l2b", [5, 1], dt, kind="ExternalInput")
    out_d = nc.dram_tensor("out", [5, B_LOC], dt, kind="ExternalOutput")

    with tile.TileContext(nc) as tc, ExitStack() as ctx:
        sing = ctx.enter_context(tc.tile_pool(name="sing", bufs=1))
        sb = ctx.enter_context(tc.tile_pool(name="sb", bufs=2))
        # PSUM: slots are per-tag; total tags*bufs across pools must be <= 8
        pM = ctx.enter_context(tc.tile_pool(name="pM", bufs=2, space="PSUM"))
        pB = ctx.enter_context(tc.tile_pool(name="pB", bufs=2, space="PSUM"))
        pE = ctx.enter_context(tc.tile_pool(name="pE", bufs=2, space="PSUM"))
        pS = ctx.enter_context(tc.tile_pool(name="pS", bufs=1, space="PSUM"))

        def load(d, shape, tag, src=None):
            t = sing.tile(shape, dt, tag=tag)
            nc.sync.dma_start(t[:], src if src is not None else d[:])
            return t

        w1_s = load(w1_d, [3, 16], "w1")
        b1_s = load(b1_d, [16, 1], "b1")
        w2_s = load(w2_d, [16, 20], "w2")
        b2_s = load(b2_d, [20, 1], "b2")
        wk_s = load(wk_d, [20, KQV_N], "wk")
        wq_s = load(wq_d, [20, KQV_N], "wq")
        wv_s = load(wv_d, [20, KQV_N], "wv")
        ck1_s = load(ck_d, [128, NODES], "ck1", ck_d[0:128, :])
        ck2_s = load(ck_d, [64, NODES], "ck2", ck_d[128:KQV_N, :])
        cq1_s = load(cq_d, [128, NODES], "cq1", cq_d[0:128, :])
        cq2_s = load(cq_d, [64, NODES], "cq2", cq_d[128:KQV_N, :])
        cv_s = load(cv_d, [NODES, KQV_N], "cv")
        # qlw/klw duplicated across both 64-partition halves so the
        # stationary slice can match the rhs base partition per head
        qlw_s = sing.tile([128, NODES], dt, tag="qlw")
        nc.sync.dma_start(qlw_s[0:D, :], qlw_d[:])
        nc.sync.dma_start(qlw_s[D:2 * D, :], qlw_d[:])
        klw_s = sing.tile([128, NODES], dt, tag="klw")
        nc.sync.dma_start(klw_s[0:D, :], klw_d[:])
        nc.sync.dma_start(klw_s[D:2 * D, :], klw_d[:])
        qklb_s = load(qklb_d, [NODES, 1], "qklb")
        alw_s = load(alw_d, [NODES, NODES], "alw")
        alb_s = load(alb_d, [NODES, 1], "alb")
        l1t_s = load(l1t_d, [D, KQV_N], "l1t")
        nls_s = load(nls_d, [1, D], "nls")
        l1b_s = load(l1b_d, [D, 1], "l1b")
        l2w_s = load(l2w_d, [D, 5], "l2w")
        l2b_s = load(l2b_d, [5, 1], "l2b")

        ones_col = sing.tile([128, 1], dt, tag="onescol")
        nc.vector.memset(ones_col[:], 1.0)
        ones_row = sing.tile([1, 128], dt, tag="onesrow")
        nc.vector.memset(ones_row[:], 1.0)

        R_K = 1.0 / (KQV_N * NODES)     # 1/9408
        R_L = 1.0 / (NODES * D)         # 1/3136

        def r3(ap):
            return ap.rearrange("p (b n) -> p b n", n=NODES)

        def chunk_body(i):
            # ---- input / 1x1 convs (feature-major [feat, (b,n)]) ----
            xt_t = sb.tile([3, COLS], dt, tag="xt")
            nc.sync.dma_start(xt_t[:], xt_d[:, ts(i, COLS)])

            ps_h1 = pM.tile([16, COLS], dt, tag="mm")
            nc.tensor.matmul(ps_h1[:], w1_s[:], xt_t[:], start=True, stop=True)
            h1 = sb.tile([16, COLS], dt, tag="h1s")
            nc.scalar.activation(h1[:], ps_h1[:], AF.Relu, bias=b1_s[:])

            ps_h2 = pM.tile([20, COLS], dt, tag="mm")
            nc.tensor.matmul(ps_h2[:], w2_s[:], h1[:], start=True, stop=True)
            h2 = sb.tile([20, COLS], dt, tag="h2s")
            nc.scalar.activation(h2[:], ps_h2[:], AF.Relu, bias=b2_s[:])

            # ---- K, Q projections, feature-major split 128+64 ----
            def proj_fm(w_s, c1_s, c2_s, tagp):
                outs = []
                for bi, (f0, fn, c_s) in enumerate(
                        [(0, 128, c1_s), (128, 64, c2_s)]):
                    ps = pM.tile([fn, COLS], dt, tag="mm")
                    nc.tensor.matmul(ps[:], w_s[:, f0:f0 + fn], h2[:],
                                     start=True, stop=True)
                    t = sb.tile([fn, COLS], dt, tag=f"{tagp}s{bi}")
                    nc.vector.tensor_add(
                        r3(t[:]), r3(ps[:]),
                        c_s[:, None, :].broadcast_to([fn, CB, NODES]))
                    outs.append(t)
                return outs

            K1, K2 = proj_fm(wk_s, ck1_s, ck2_s, "k")
            Q1, Q2 = proj_fm(wq_s, cq1_s, cq2_s, "q")

            # ---- V token-major [c, (b, h*64+d)] ----
            v_tm = sb.tile([NODES, CB * KQV_N], dt, tag="vtm")
            for b in range(CB):
                ps_v = pM.tile([NODES, KQV_N], dt, tag="mm")
                nc.tensor.matmul(ps_v[:], h2[:, b * NODES:(b + 1) * NODES],
                                 wv_s[:], start=True, stop=True)
                nc.vector.tensor_add(
                    v_tm[:, b * KQV_N:(b + 1) * KQV_N], ps_v[:], cv_s[:])

            # ---- LN stats (sum, sumsq per batch elem) ----
            def stats_fm(T1, T2, tagp):
                T1q = sb.tile([128, COLS], dt, tag=f"{tagp}sq1")
                nc.scalar.activation(T1q[:], T1[:], AF.Square)
                T2q = sb.tile([64, COLS], dt, tag=f"{tagp}sq2")
                nc.scalar.activation(T2q[:], T2[:], AF.Square)
                r1 = sb.tile([128, 2 * CB], dt, tag=f"{tagp}r1")
                r2 = sb.tile([64, 2 * CB], dt, tag=f"{tagp}r2")
                for dst, src in ((r1[:, 0:CB], T1), (r1[:, CB:2 * CB], T1q),
                                 (r2[:, 0:CB], T2), (r2[:, CB:2 * CB], T2q)):
                    nc.vector.tensor_reduce(
                        dst, r3(src[:]), mybir.AxisListType.X, ALU.add)
                ps_st = pS.tile([1, 2 * CB], dt, tag="row")
                nc.tensor.matmul(ps_st[:], ones_col[:], r1[:],
                                 start=True, stop=False)
                nc.tensor.matmul(ps_st[:], ones_col[0:64, :], r2[:],
                                 start=False, stop=True)
                return ps_st

            def stats_tm(Tv):
                Tq = sb.tile([NODES, CB * KQV_N], dt, tag="vsq")
                nc.scalar.activation(Tq[:], Tv[:], AF.Square)
                rv = sb.tile([NODES, 2 * CB], dt, tag="vr")
                for dst, src in ((rv[:, 0:CB], Tv), (rv[:, CB:2 * CB], Tq)):
                    nc.vector.tensor_reduce(
                        dst, src[:].rearrange("p (b f) -> p b f", f=KQV_N),
                        mybir.AxisListType.X, ALU.add)
                ps_st = pS.tile([1, 2 * CB], dt, tag="row")
                nc.tensor.matmul(ps_st[:], ones_col[0:NODES, :], rv[:],
                                 start=True, stop=True)
                return ps_st

            def finalize_stats(ps_st, recip_n, tagp):
                rows = sb.tile([1, 4 * CB], dt, tag=f"{tagp}rows")
                m = rows[:, 0:CB]
                rs = rows[:, CB:2 * CB]
                mrs = rows[:, 2 * CB:3 * CB]
                sq = rows[:, 3 * CB:4 * CB]
                nc.scalar.activation(m, ps_st[:, 0:CB], AF.Copy, scale=recip_n)
                nc.scalar.activation(sq, ps_st[:, CB:2 * CB], AF.Copy,
                                     scale=recip_n)
                m2 = sb.tile([1, CB], dt, tag=f"{tagp}m2")
                nc.scalar.activation(m2[:], m, AF.Square)
                var = sb.tile([1, CB], dt, tag=f"{tagp}var")
                nc.vector.scalar_tensor_tensor(
                    var[:], sq, EPS, m2[:], ALU.add, ALU.subtract)
                sd = sb.tile([1, CB], dt, tag=f"{tagp}sd")
                nc.scalar.activation(sd[:], var[:], AF.Sqrt)
                nc.vector.reciprocal(rs, sd[:])
                nc.vector.tensor_mul(mrs, m, rs)
                return rs, mrs

            st_k = stats_fm(K1, K2, "k")
            rs_k, mrs_k = finalize_stats(st_k, R_K, "k")
            st_q = stats_fm(Q1, Q2, "q")
            rs_q, mrs_q = finalize_stats(st_q, R_K, "q")
            st_v = stats_tm(v_tm)
            rs_v, mrs_v = finalize_stats(st_v, R_K, "v")

            # ---- normalize K, Q: Kn = K*rs - m*rs ----
            def bc_pair(rs, mrs, p):
                b_rs = pB.tile([p, CB], dt, tag="bc")
                nc.tensor.matmul(b_rs[:], ones_row[:, 0:p], rs,
                                 start=True, stop=True)
                b_mrs = pB.tile([p, CB], dt, tag="bc")
                nc.tensor.matmul(b_mrs[:], ones_row[:, 0:p], mrs,
                                 start=True, stop=True)
                return b_rs, b_mrs

            def apply_norm(T1, T2, b_rs, b_mrs, tagp):
                n1 = sb.tile([128, COLS], dt, tag=f"{tagp}n1")
                n2 = sb.tile([64, COLS], dt, tag=f"{tagp}n2")
                for dst, src, p in ((n1, T1, 128), (n2, T2, 64)):
                    nc.vector.tensor_tensor(
                        r3(dst[:]), r3(src[:]),
                        b_rs[0:p, :, None].broadcast_to([p, CB, NODES]),
                        ALU.mult)
                    nc.vector.tensor_tensor(
                        r3(dst[:]), r3(dst[:]),
                        b_mrs[0:p, :, None].broadcast_to([p, CB, NODES]),
                        ALU.subtract)
                return n1, n2

            bk_rs, bk_mrs = bc_pair(rs_k, mrs_k, 128)
            K1n, K2n = apply_norm(K1, K2, bk_rs, bk_mrs, "k")
            bq_rs, bq_mrs = bc_pair(rs_q, mrs_q, 128)
            Q1n, Q2n = apply_norm(Q1, Q2, bq_rs, bq_mrs, "q")

            # ---- attention scores + softmax per head ----
            a_n = sb.tile([NODES, 3 * COLS], dt, tag="an")
            for h in range(NHEADS):
                if h == 0:
                    qh, kh = Q1n[0:64, :], K1n[0:64, :]
                    qw, kw = qlw_s[0:D, :], klw_s[0:D, :]
                elif h == 1:
                    qh, kh = Q1n[64:128, :], K1n[64:128, :]
                    qw, kw = qlw_s[D:2 * D, :], klw_s[D:2 * D, :]
                else:
                    qh, kh = Q2n[:], K2n[:]
                    qw, kw = qlw_s[0:D, :], klw_s[0:D, :]
                ps_sq = pM.tile([NODES, COLS], dt, tag="mm")
                nc.tensor.matmul(ps_sq[:], qw, qh, start=True, stop=True)
                ps_sk = pM.tile([NODES, COLS], dt, tag="mm")
                nc.tensor.matmul(ps_sk[:], kw, kh, start=True, stop=True)
                u = sb.tile([NODES, COLS], dt, tag="u")
                nc.scalar.activation(u[:], ps_sq[:], AF.Identity,
                                     bias=qklb_s[:])
                x = sb.tile([NODES, COLS], dt, tag="x")
                nc.vector.tensor_add(x[:], u[:], ps_sk[:])
                # elu(x) = max(x, exp(min(x,0)) - 1)
                mn = sb.tile([NODES, COLS], dt, tag="mn")
                nc.vector.tensor_scalar_min(mn[:], x[:], 0.0)
                ex = sb.tile([NODES, COLS], dt, tag="ex")
                nc.scalar.activation(ex[:], mn[:], AF.Exp)
                a_h = sb.tile([NODES, COLS], dt, tag="ah")
                nc.vector.scalar_tensor_tensor(
                    a_h[:], ex[:], -1.0, x[:], ALU.add, ALU.max)
                # A2 = alw^T @ a_h (+ alb), exp, softmax over partitions
                ps_a2 = pM.tile([NODES, COLS], dt, tag="mm")
                nc.tensor.matmul(ps_a2[:], alw_s[:], a_h[:],
                                 start=True, stop=True)
                expa = sb.tile([NODES, COLS], dt, tag="expa")
                nc.scalar.activation(expa[:], ps_a2[:], AF.Exp, bias=alb_s[:])
                ps_den = pS.tile([1, COLS], dt, tag="row")
                nc.tensor.matmul(ps_den[:], ones_col[0:NODES, :], expa[:],
                                 start=True, stop=True)
                rdr = sb.tile([1, COLS], dt, tag="rdr")
                nc.vector.reciprocal(rdr[:], ps_den[:])
                ps_bcd = pM.tile([NODES, COLS], dt, tag="mm")
                nc.tensor.matmul(ps_bcd[:], ones_row[:, 0:NODES], rdr[:],
                                 start=True, stop=True)
                nc.vector.tensor_tensor(
                    a_n[:, h * COLS:(h + 1) * COLS], expa[:], ps_bcd[:],
                    ALU.mult)

            # ---- E = A_n^T-contraction with V, per (b,h), fm output ----
            e_sb = sb.tile([D, 3 * COLS], dt, tag="esb")
            for h in range(NHEADS):
                ps_e = pE.tile([D, COLS], dt, tag="e")
                for b in range(CB):
                    nc.tensor.matmul(
                        ps_e[:, b * NODES:(b + 1) * NODES],
                        v_tm[:, b * KQV_N + h * D: b * KQV_N + (h + 1) * D],
                        a_n[:, h * COLS + b * NODES:
                            h * COLS + (b + 1) * NODES],
                        start=True, stop=True)
                nc.scalar.activation(e_sb[:, h * COLS:(h + 1) * COLS],
                                     ps_e[:], AF.Copy)

            # ---- E2 = lin1^T E (acc over h), V-LN fixup, relu ----
            ps_e2 = pM.tile([D, COLS], dt, tag="mm")
            for h in range(NHEADS):
                nc.tensor.matmul(ps_e2[:], l1t_s[:, h * D:(h + 1) * D],
                                 e_sb[:, h * COLS:(h + 1) * COLS],
                                 start=(h == 0), stop=(h == 2))
            e2s = sb.tile([D, COLS], dt, tag="e2s")
            nc.scalar.activation(e2s[:], ps_e2[:], AF.Copy)
            bv_rs = pB.tile([D, CB], dt, tag="bc")
            nc.tensor.matmul(bv_rs[:], ones_row[:, 0:D], rs_v,
                             start=True, stop=True)
            bv_add = pB.tile([D, CB], dt, tag="bc")
            nc.tensor.matmul(bv_add[:], nls_s[:], mrs_v, start=True, stop=True)
            t_fix = sb.tile([D, COLS], dt, tag="tfix")
            nc.vector.tensor_tensor(
                r3(t_fix[:]), r3(e2s[:]),
                bv_rs[:, :, None].broadcast_to([D, CB, NODES]), ALU.mult)
            t2 = sb.tile([D, COLS], dt, tag="t2")
            nc.vector.scalar_tensor_tensor(
                r3(t2[:]), r3(t_fix[:]), l1b_s[:],
                bv_add[:, :, None].broadcast_to([D, CB, NODES]),
                ALU.add, ALU.add)
            e3 = sb.tile([D, COLS], dt, tag="e3")
            nc.scalar.activation(e3[:], t2[:], AF.Relu)

            # ---- final LN (no affine) + max over n + lin2 + elu ----
            e3q = sb.tile([D, COLS], dt, tag="e3q")
            nc.scalar.activation(e3q[:], e3[:], AF.Square)
            re = sb.tile([D, 2 * CB], dt, tag="re")
            nc.vector.tensor_reduce(
                re[:, 0:CB], r3(e3[:]), mybir.AxisListType.X, ALU.add)
            nc.vector.tensor_reduce(
                re[:, CB:2 * CB], r3(e3q[:]), mybir.AxisListType.X, ALU.add)
            emax = sb.tile([D, CB], dt, tag="emax")
            nc.vector.tensor_reduce(
                emax[:], r3(e3[:]), mybir.AxisListType.X, ALU.max)
            ps_stl = pS.tile([1, 2 * CB], dt, tag="row")
            nc.tensor.matmul(ps_stl[:], ones_col[0:D, :], re[:],
                             start=True, stop=True)
            rs_l, mrs_l = finalize_stats(ps_stl, R_L, "l")
            bl_rs, bl_mrs = bc_pair(rs_l, mrs_l, D)
            e4 = sb.tile([D, CB], dt, tag="e4")
            nc.vector.tensor_tensor(e4[:], emax[:], bl_rs[:], ALU.mult)
            e5 = sb.tile([D, CB], dt, tag="e5")
            nc.vector.tensor_tensor(e5[:], e4[:], bl_mrs[:], ALU.subtract)

            ps_o = pS.tile([5, CB], dt, tag="row")
            nc.tensor.matmul(ps_o[:], l2w_s[:], e5[:], start=True, stop=True)
            so = sb.tile([5, CB], dt, tag="so")
            nc.scalar.activation(so[:], ps_o[:], AF.Identity, bias=l2b_s[:])
            mno = sb.tile([5, CB], dt, tag="mno")
            nc.vector.tensor_scalar_min(mno[:], so[:], 0.0)
            exo = sb.tile([5, CB], dt, tag="exo")
            nc.scalar.activation(exo[:], mno[:], AF.Exp)
            oc = sb.tile([5, CB], dt, tag="oc")
            nc.vector.scalar_tensor_tensor(
                oc[:], exo[:], -1.0, so[:], ALU.add, ALU.max)
            nc.sync.dma_start(out_d[:, ts(i, CB)], oc[:])

        with tc.For_i(0, NCHUNK, 1) as i:
            chunk_body(i)

    nc.finalize()
    return nc


def _prep_consts(conv1_w, conv1_b, conv2_w, conv2_b,
                 k_proj_w, k_proj_b, q_proj_w, q_proj_b, v_proj_w, v_proj_b,
                 k_lin_w, k_lin_b, q_lin_w, q_lin_b, a_lin_w, a_lin_b,
                 lin1_w, lin1_b, lin2_w, lin2_b):
    f32 = np.float32
    C = lambda a: np.ascontiguousarray(np.asarray(a, f32))
    xc = np.arange(7, dtype=f32) / 7
    yc = np.arange(7, dtype=f32) / 7
    coords = np.stack([np.tile(xc, 7), np.repeat(yc, 7)], axis=1)  # [49,2]
    return {
        "w1t": C(np.asarray(conv1_w, f32).T),
        "b1": C(np.asarray(conv1_b, f32)[:, None]),
        "w2t": C(np.asarray(conv2_w, f32).T),
        "b2": C(np.asarray(conv2_b, f32)[:, None]),
        "wk": C(np.asarray(k_proj_w, f32)[:20]),
        "wq": C(np.asarray(q_proj_w, f32)[:20]),
        "wv": C(np.asarray(v_proj_w, f32)[:20]),
        "ck": C((coords @ np.asarray(k_proj_w, f32)[20:22]
                 + np.asarray(k_proj_b, f32)[None, :]).T),
        "cq": C((coords @ np.asarray(q_proj_w, f32)[20:22]
                 + np.asarray(q_proj_b, f32)[None, :]).T),
        "cvtm": C(coords @ np.asarray(v_proj_w, f32)[20:22]
                  + np.asarray(v_proj_b, f32)[None, :]),
        "qlw": C(q_lin_w),
        "klw": C(k_lin_w),
        "qklb": C((np.asarray(q_lin_b, f32)
                   + np.asarray(k_lin_b, f32))[:, None]),
        "alw": C(a_lin_w),
        "alb": C(np.asarray(a_lin_b, f32)[:, None]),
        "l1t": C(np.hstack([np.asarray(lin1_w, f32)[h * 64:(h + 1) * 64]
                            for h in range(3)])),
        "negl1sum": C(-np.asarray(lin1_w, f32).sum(axis=0)[None, :]),
        "l1b": C(np.asarray(lin1_b, f32)[:, None]),
        "l2w": C(lin2_w),
        "l2b": C(np.asarray(lin2_b, f32)[:, None]),
    }


def _numpy_fallback(x, conv1_w, conv1_b, conv2_w, conv2_b,
                    k_proj_w, k_proj_b, q_proj_w, q_proj_b,
                    v_proj_w, v_proj_b,
                    k_norm_g, k_norm_b, q_norm_g, q_norm_b,
                    v_norm_g, v_norm_b,
                    k_lin_w, k_lin_b, q_lin_w, q_lin_b, a_lin_w, a_lin_b,
                    lin1_w, lin1_b, lin2_w, lin2_b):
    f32 = np.float32
    x = np.asarray(x, f32)
    b = x.shape[0]
    h = np.maximum(np.einsum('bchw,oc->bohw', x, np.asarray(conv1_w, f32))
                   + np.asarray(conv1_b, f32)[None, :, None, None], 0)
    h = np.maximum(np.einsum('bchw,oc->bohw', h, np.asarray(conv2_w, f32))
                   + np.asarray(conv2_b, f32)[None, :, None, None], 0)
    xc = np.tile((np.arange(7, dtype=f32) / 7)[None, :], (7, 1))
    yc = np.tile((np.arange(7, dtype=f32) / 7)[:, None], (1, 7))
    coords = np.broadcast_to(np.stack([xc, yc], 0)[None], (b, 2, 7, 7))
    h = np.concatenate([h, coords], axis=1)
    tokens = h.transpose(0, 2, 3, 1).reshape(b, NODES, -1)

    def ln(t, axes, g, beta):
        m = t.mean(axis=axes, keepdims=True)
        v = t.var(axis=axes, keepdims=True)
        return (t - m) / np.sqrt(v + EPS) * g + beta

    def proj(w, bias, g, beta):
        p = tokens @ np.asarray(w, f32) + np.asarray(bias, f32)
        p = p.reshape(b, NODES, NHEADS, D).transpose(0, 2, 1, 3)
        return ln(p, (1, 2, 3), np.asarray(g, f32), np.asarray(beta, f32))

    K = proj(k_proj_w, k_proj_b, k_norm_g, k_norm_b)
    Q = proj(q_proj_w, q_proj_b, q_norm_g, q_norm_b)
    V = proj(v_proj_w, v_proj_b, v_norm_g, v_norm_b)

    def elu(t):
        return np.where(t > 0, t, np.expm1(np.minimum(t, 0.0)))

    A = elu((Q @ np.asarray(q_lin_w, f32) + np.asarray(q_lin_b, f32))
            + (K @ np.asarray(k_lin_w, f32) + np.asarray(k_lin_b, f32)))
    A = A @ np.asarray(a_lin_w, f32) + np.asarray(a_lin_b, f32)
    A = A - A.max(axis=-1, keepdims=True)
    np.exp(A, out=A)
    A /= A.sum(axis=-1, keepdims=True)
    E = A @ V
    E = E.transpose(0, 2, 1, 3).reshape(b, NODES, NHEADS * D)
    E = np.maximum(E @ np.asarray(lin1_w, f32) + np.asarray(lin1_b, f32), 0.0)
    m = E.mean(axis=(1, 2), keepdims=True)
    v = E.var(axis=(1, 2), keepdims=True)
    E = (E - m) / np.sqrt(v + EPS)
    E = E.max(axis=1)
    out = E @ np.asarray(lin2_w, f32) + np.asarray(lin2_b, f32)
    return elu(out).astype(np.float32)


def kernel(x, conv1_w, conv1_b, conv2_w, conv2_b,
           k_proj_w, k_proj_b, q_proj_w, q_proj_b, v_proj_w, v_proj_b,
           k_norm_g, k_norm_b, q_norm_g, q_norm_b, v_norm_g, v_norm_b,
           k_lin_w, k_lin_b, q_lin_w, q_lin_b, a_lin_w, a_lin_b,
           lin1_w, lin1_b, lin2_w, lin2_b):
    f32 = np.float32
    x = np.asarray(x, f32)
    b = x.shape[0]
    general = (b != B
               or not (np.all(np.asarray(k_norm_g) == 1)
                       and np.all(np.asarray(q_norm_g) == 1)
                       and np.all(np.asarray(v_norm_g) == 1)
                       and np.all(np.asarray(k_norm_b) == 0)
                       and np.all(np.asarray(q_norm_b) == 0)
                       and np.all(np.asarray(v_norm_b) == 0)))
    if general:
        return _numpy_fallback(
            x, conv1_w, conv1_b, conv2_w, conv2_b,
            k_proj_w, k_proj_b, q_proj_w, q_proj_b, v_proj_w, v_proj_b,
            k_norm_g, k_norm_b, q_norm_g, q_norm_b, v_norm_g, v_norm_b,
            k_lin_w, k_lin_b, q_lin_w, q_lin_b, a_lin_w, a_lin_b,
            lin1_w, lin1_b, lin2_w, lin2_b)

    if "nc" not in _CACHE:
        _CACHE["nc"] = _build_nc()
    nc = _CACHE["nc"]

    consts = _prep_consts(
        conv1_w, conv1_b, conv2_w, conv2_b,
        k_proj_w, k_proj_b, q_proj_w, q_proj_b, v_proj_w, v_proj_b,
        k_lin_w, k_lin_b, q_lin_w, q_lin_b, a_lin_w, a_lin_b,
        lin1_w, lin1_b, lin2_w, lin2_b)

    xr = x.reshape(B, 3, NODES)
    in_maps = []
    for c in range(N_CORES):
        xs = xr[c * B_LOC:(c + 1) * B_LOC]
        xt = np.ascontiguousarray(
            xs.transpose(1, 0, 2).reshape(3, B_LOC * NODES), f32)
        in_maps.append({"xt": xt, **consts})

    res = run_bass_kernel_spmd(nc, in_maps, list(range(N_CORES)))
    out = np.concatenate(
        [res.results[c]["out"].T for c in range(N_CORES)], axis=0)
    return np.ascontiguousarray(out, f32)
